# revision 35
# baseline (speedup 1.0000x reference)
"""nn_MultiHeadAttention TRN2 kernel: 8-core tensor-parallel (2 heads/core).

Self-contained: builds and compiles the Bass/Tile SPMD program on first call,
shards the full inputs per-core on the host, runs via run_bass_kernel_spmd,
and concatenates the per-core sequence-block outputs into the full output.

v3 design (per core, 2 heads of 16, head_dim 64, S=4096, D=1024):
  - feature-major xT [D,S]; q/k projected with RoPE-permuted transposed
    weight shards laid out so the rotary partner row sits in the same
    32-partition quadrant: rotation = one DVE stream_shuffle (16-row swap)
    instead of SBUF-SBUF DMAs.
  - v seq-major into a 4-D v_aug tile [128, 32, 2, 65] whose 65th column is
    1.0 (softmax denominator produced by the PV matmul directly).
  - projection and attention emission interleaved per 512-block so the PE
    pipeline never drains (DVFS: sustained activity ramps PE 1.2->2.4 GHz).
  - flash attention on transposed score tiles: per Q-block the heads'
    exp-groups alternate and PV emission is skewed one group behind scores,
    keeping TensorE busy while ScalarE computes exp.
  - normalization fully decoupled from the PE: outT rows copied to SBUF,
    reciprocal_approx_fast + gpsimd partition_broadcast + in-place DVE
    multiply; only the A2A staging DMA waits on it.
  - per-Q-block staging DMAs feed one AllToAll (head-split -> seq-split);
    final projection against full Wo.T; each core emits out[512, 1024] f32.
"""

from contextlib import ExitStack

import numpy as np
import ml_dtypes

import concourse.tile as tile
from concourse import bacc, mybir
from concourse.bass_utils import run_bass_kernel_spmd

F32 = mybir.dt.float32
BF16 = mybir.dt.bfloat16

S = 4096
D = 1024
HD = 64
N_CORES = 8
KT = 128
BQ = 512
CHUNK = S // N_CORES
NFT = D // 128
NKT = S // KT
NQB = S // BQ
G = 2

# stream_shuffle mask: swap 16-row halves within each 32-partition quadrant
SWAP16 = [16 + i for i in range(16)] + list(range(16))

# Schraudolph exp in bf16 bit-space: bf16_bits(exp(0.125*s)) ~= A*s + B.
# Applied on the DVE to mask-free score groups to offload the scalar engine.
A_SCH = 128 * 0.125 / 0.6931471805599453
B_SCH = 16248.5
OFF_K = 3  # offload every OFF_K-th mask-free group to the DVE


def _build():
    nc = bacc.Bacc("TRN2", target_bir_lowering=False, debug=False, num_devices=N_CORES)

    xT = nc.dram_tensor("xT", [D, S], BF16, kind="ExternalInput")
    wq = nc.dram_tensor("wq", [128, NFT * 128], BF16, kind="ExternalInput")
    wk = nc.dram_tensor("wk", [128, NFT * 128], BF16, kind="ExternalInput")
    wv = nc.dram_tensor("wv", [128, NFT * 128], BF16, kind="ExternalInput")
    wo = nc.dram_tensor("wo", [128, NFT * D], BF16, kind="ExternalInput")
    cosP = nc.dram_tensor("cosP", [128, S], BF16, kind="ExternalInput")
    sinN = nc.dram_tensor("sinN", [128, S], BF16, kind="ExternalInput")
    lu = nc.dram_tensor("lu", [128, 128], BF16, kind="ExternalInput")
    out = nc.dram_tensor("out", [CHUNK, D], F32, kind="ExternalOutput")

    a2a_in = nc.dram_tensor("a2a_in", [N_CORES * 128, CHUNK], BF16)
    a2a_out = nc.dram_tensor("a2a_out", [N_CORES * 128, CHUNK], BF16)

    with tile.TileContext(nc) as tc, ExitStack() as ctx:
        sb = ctx.enter_context(tc.tile_pool(name="sb", bufs=1))
        xt_s = [sb.tile([128, S], BF16, tag=f"xt{t}", name=f"xt{t}") for t in range(NFT)]
        wq_s = sb.tile([128, NFT * 128], BF16, tag="wq", name="wq_s")
        wk_s = sb.tile([128, NFT * 128], BF16, tag="wk", name="wk_s")
        wv_s = sb.tile([128, NFT * 128], BF16, tag="wv", name="wv_s")
        wo_s = sb.tile([128, NFT, D], BF16, tag="wo", name="wo_s")
        cos_s = sb.tile([128, S], BF16, tag="cos", name="cos_s")
        sin_s = sb.tile([128, S], BF16, tag="sin", name="sin_s")
        lu_s = sb.tile([128, 128], BF16, tag="lu", name="lu_s")
        qA = sb.tile([128, S], BF16, tag="qA", name="qA")
        kA = sb.tile([128, S], BF16, tag="kA", name="kA")
        qT = sb.tile([128, S], BF16, tag="qT", name="qT")
        kT_ = sb.tile([128, S], BF16, tag="kT", name="kT_")
        v_aug = sb.tile([128, NKT, 2, 65], BF16, tag="vaug", name="v_aug")
        attnT = sb.tile([128, S], BF16, tag="attnT", name="attnT")
        aT = [sb.tile([128, CHUNK], BF16, tag=f"aT{t}", name=f"aT{t}") for t in range(NFT)]

        nc.vector.memset(v_aug[:], 1.0)

        nc.sync.dma_start(wq_s[:], wq[:, :])
        nc.sync.dma_start(wk_s[:], wk[:, :])
        nc.sync.dma_start(wv_s[:], wv[:, :])
        # x + rope tables chunk-interleaved in priority order, on the idle
        # scalar queue; wo (needed only for the output projection) last
        for nb2 in range(NQB // 2):
            c = slice(1024 * nb2, 1024 * (nb2 + 1))
            for t in range(NFT):
                nc.sync.dma_start(xt_s[t][:, c], xT[128 * t : 128 * (t + 1), c])
            nc.sync.dma_start(cos_s[:, c], cosP[:, c])
            nc.sync.dma_start(sin_s[:, c], sinN[:, c])
        nc.sync.dma_start(lu_s[:], lu[:, :])
        nc.sync.dma_start(
            wo_s[:], wo[:, :].rearrange("p (t c) -> p t c", t=NFT)
        )
        warm_i = sb.tile([1, 64], F32, tag="warm_i", name="warm_i")
        warm_o = sb.tile([128, 64], F32, tag="warm_o", name="warm_o")
        nc.vector.memset(warm_i[:], 1.0)
        nc.gpsimd.partition_broadcast(warm_o[:], warm_i[:])

        psc = ctx.enter_context(tc.tile_pool(name="psc", bufs=3, space="PSUM"))
        ppv = ctx.enter_context(tc.tile_pool(name="ppv", bufs=2, space="PSUM"))

        def proj_block(nb):
            c = slice(BQ * nb, BQ * (nb + 1))
            qp = psc.tile([128, BQ], F32, tag="sc", name="qp")
            for t in range(NFT):
                nc.tensor.matmul(
                    qp[:], wq_s[:, 128 * t : 128 * (t + 1)], xt_s[t][:, c],
                    start=(t == 0), stop=(t == NFT - 1),
                )
            kp = psc.tile([128, BQ], F32, tag="sc", name="kp")
            for t in range(NFT):
                nc.tensor.matmul(
                    kp[:], wk_s[:, 128 * t : 128 * (t + 1)], xt_s[t][:, c],
                    start=(t == 0), stop=(t == NFT - 1),
                )
            nc.vector.tensor_copy(qA[:, c], qp[:])
            nc.vector.tensor_copy(kA[:, c], kp[:])
            vp = psc.tile([128, BQ], F32, tag="sc", name="vp")
            for u in range(4):
                st = slice(BQ * nb + 128 * u, BQ * nb + 128 * (u + 1))
                for t in range(NFT):
                    nc.tensor.matmul(
                        vp[:, 128 * u : 128 * (u + 1)], xt_s[t][:, st],
                        wv_s[:, 128 * t : 128 * (t + 1)],
                        start=(t == 0), stop=(t == NFT - 1),
                    )
            nc.vector.tensor_copy(
                v_aug[:, 4 * nb : 4 * (nb + 1), :, 0:64],
                vp[:].rearrange("p (u h c) -> p u h c", u=4, h=2, c=64),
            )
            qBt = sb.tile([128, BQ], BF16, tag="qB", name="qBt", bufs=2)
            kBt = sb.tile([128, BQ], BF16, tag="kB", name="kBt", bufs=2)
            nc.vector.stream_shuffle(qBt[:], qA[:, c], SWAP16)
            nc.vector.stream_shuffle(kBt[:], kA[:, c], SWAP16)
            for A, B, R in ((qA, qBt, qT), (kA, kBt, kT_)):
                nc.vector.tensor_mul(R[:, c], A[:, c], cos_s[:, c])
                nc.vector.tensor_mul(B[:], B[:], sin_s[:, c])
                nc.vector.tensor_add(R[:, c], R[:, c], B[:])

        def attn_block(Q):
            q0 = BQ * Q
            n_jt = 4 * (Q + 1)
            jts = list(range(n_jt))
            groups = [jts[i : i + G] for i in range(0, n_jt, G)]
            seq = [(h, g) for g in groups for h in range(2)]
            outT = {}

            def emit_pv(h, g, ex):
                if g[0] == 0:
                    outT[h] = ppv.tile([65, BQ], F32, tag="pv", name=f"outT{h}")
                for idx, jt in enumerate(g):
                    trim = max(0, KT * jt - q0)
                    nc.tensor.matmul(
                        outT[h][:, trim:BQ],
                        v_aug[:, jt, h, :],
                        ex[:, BQ * idx + trim : BQ * (idx + 1)],
                        start=(jt == 0),
                        stop=(jt == n_jt - 1),
                    )

            pending = None
            for h, g in seq:
                hb = 64 * h
                sc = psc.tile([128, BQ * len(g)], F32, tag="sc", name="sc")
                for idx, jt in enumerate(g):
                    trim = max(0, KT * jt - q0)
                    nc.tensor.matmul(
                        sc[:, BQ * idx + trim : BQ * (idx + 1)],
                        kT_[hb : hb + 64, KT * jt : KT * (jt + 1)],
                        qT[hb : hb + 64, q0 + trim : q0 + BQ],
                        start=True, stop=True,
                    )
                for idx, jt in enumerate(g):
                    if KT * jt >= q0:
                        trim = KT * jt - q0
                        nc.vector.tensor_add(
                            sc[:, BQ * idx + trim : BQ * idx + trim + 128],
                            sc[:, BQ * idx + trim : BQ * idx + trim + 128],
                            lu_s[:],
                        )
                ex = sb.tile([128, G * BQ], BF16, tag="expT", name="expT", bufs=6)
                t0 = max(0, KT * g[0] - q0)
                nc.scalar.activation(
                    ex[:, t0 : BQ * len(g)], sc[:, t0 : BQ * len(g)],
                    mybir.ActivationFunctionType.Exp, scale=0.125,
                )
                if pending is not None:
                    emit_pv(*pending)
                pending = (h, g, ex)
            emit_pv(*pending)

            return outT

        def norm_block(Q, outT):
            q0 = BQ * Q
            den_s = sb.tile([1, 2 * BQ], F32, tag="dens", name="dens", bufs=2)
            den_f = sb.tile([1, 2 * BQ], F32, tag="denf", name="denf", bufs=2)
            bc_sb = sb.tile([128, 2 * BQ], F32, tag="bc", name="bc", bufs=2)
            for h in range(2):
                hb = 64 * h
                nc.vector.tensor_copy(den_s[:, BQ * h : BQ * (h + 1)], outT[h][64:65, :])
                nc.vector.tensor_copy(attnT[hb : hb + 64, q0 : q0 + BQ], outT[h][0:64, :])
            nc.vector.reciprocal_approx_fast(den_f[:], den_s[:])
            nc.gpsimd.partition_broadcast(bc_sb[:], den_f[:])
            for h in range(2):
                hb = 64 * h
                nc.vector.tensor_mul(
                    attnT[hb : hb + 64, q0 : q0 + BQ],
                    attnT[hb : hb + 64, q0 : q0 + BQ],
                    bc_sb[hb : hb + 64, BQ * h : BQ * (h + 1)],
                )
            nc.sync.dma_start(
                a2a_in[128 * Q : 128 * (Q + 1), :], attnT[:, q0 : q0 + BQ]
            )

        pending_norm = None

        def run_attn(Q):
            nonlocal pending_norm
            o = attn_block(Q)
            if pending_norm is not None:
                norm_block(*pending_norm)
            pending_norm = (Q, o)

        LEAD = 6
        for nb in range(NQB):
            proj_block(nb)
            if nb >= LEAD:
                run_attn(nb - LEAD)
        for Q in range(NQB - LEAD, NQB):
            run_attn(Q)
        norm_block(*pending_norm)



        nc.gpsimd.collective_compute(
            "AllToAll",
            mybir.AluOpType.bypass,
            replica_groups=[list(range(N_CORES))],
            ins=[a2a_in.ap().opt()],
            outs=[a2a_out.ap().opt()],
        )
        for t in range(NFT):
            nc.sync.dma_start(aT[t][:], a2a_out[128 * t : 128 * (t + 1), :])

        for it in range(CHUNK // 128):
            for oh in range(D // 512):
                p = psc.tile([128, 512], F32, tag="sc", name="p_o")
                for t in range(NFT):
                    nc.tensor.matmul(
                        p[:],
                        aT[t][:, 128 * it : 128 * (it + 1)],
                        wo_s[:, t, 512 * oh : 512 * (oh + 1)],
                        start=(t == 0), stop=(t == NFT - 1),
                    )
                ot = sb.tile([128, 512], F32, tag="oflush", name="ot", bufs=2)
                nc.scalar.copy(ot[:], p[:])
                nc.sync.dma_start(
                    out[128 * it : 128 * (it + 1), 512 * oh : 512 * (oh + 1)], ot[:]
                )

    nc.compile()
    return nc


def _host_prep(x, Wq, Wk, Wv, Wo):
    bf = ml_dtypes.bfloat16
    # quadrant-local RoPE layout: within each head (64 rows = 2 quadrants of
    # 32), quadrant q holds pairs [16q:16q+16): rows 32q+j = dim 2(16q+j)
    # (even), rows 32q+16+j = dim 2(16q+j)+1 (odd). The rotary partner of a
    # row is then 16 rows away inside the same quadrant (stream_shuffle).
    perm = np.empty(HD, dtype=np.int64)
    pair = np.empty(HD, dtype=np.int64)
    sign = np.empty(HD, dtype=np.float32)
    for q in range(2):
        for j in range(16):
            perm[32 * q + j] = 2 * (16 * q + j)
            perm[32 * q + 16 + j] = 2 * (16 * q + j) + 1
            pair[32 * q + j] = 16 * q + j
            pair[32 * q + 16 + j] = 16 * q + j
            sign[32 * q + j] = -1.0
            sign[32 * q + 16 + j] = 1.0

    inv_freq = 1.0 / (10000.0 ** (np.arange(0, HD, 2, dtype=np.float32) / HD))
    fr = np.outer(np.arange(S, dtype=np.float32), inv_freq)
    cosA = np.cos(fr).T  # [32, S] per pair index
    sinA = np.sin(fr).T
    cosH = cosA[pair]                      # [64, S]
    sinH = sinA[pair] * sign[:, None]      # [64, S]
    cosP = np.tile(cosH, (2, 1)).astype(bf)
    sinN = np.tile(sinH, (2, 1)).astype(bf)
    lu = np.tril(np.full((128, 128), -400.0, np.float32), k=-1).astype(bf)

    xT = np.ascontiguousarray(x.reshape(S, D).T).astype(bf)

    def pretile(wT, width):
        # [D, width] feature-major -> [128, NFT*width]: row p holds slab
        # chunks t at columns [t*width:(t+1)*width]
        return np.ascontiguousarray(
            wT.reshape(NFT, 128, width).transpose(1, 0, 2).reshape(128, NFT * width)
        )

    woT = pretile(np.ascontiguousarray(np.asarray(Wo, np.float32).T), D).astype(bf)

    in_maps = []
    for c in range(N_CORES):
        rows = np.concatenate([128 * c + 64 * h + perm for h in range(2)])
        in_maps.append(
            {
                "xT": xT,
                "wq": pretile(np.asarray(Wq, np.float32)[rows].T, 128).astype(bf),
                "wk": pretile(np.asarray(Wk, np.float32)[rows].T, 128).astype(bf),
                "wv": pretile(
                    np.asarray(Wv, np.float32)[128 * c : 128 * (c + 1)].T, 128
                ).astype(bf),
                "wo": woT,
                "cosP": cosP,
                "sinN": sinN,
                "lu": lu,
            }
        )
    return in_maps


_NC_CACHE = None


def kernel(x, Wq, Wk, Wv, Wo):
    global _NC_CACHE
    if _NC_CACHE is None:
        _NC_CACHE = _build()
    nc = _NC_CACHE
    in_maps = _host_prep(
        np.asarray(x, np.float32),
        np.asarray(Wq, np.float32),
        np.asarray(Wk, np.float32),
        np.asarray(Wv, np.float32),
        np.asarray(Wo, np.float32),
    )
    res = run_bass_kernel_spmd(nc, in_maps, core_ids=list(range(N_CORES)))
    full = np.concatenate([res.results[c]["out"] for c in range(N_CORES)], axis=0)
    return full.reshape(1, S, D).astype(np.float32)
